# revision 2
# baseline (speedup 1.0000x reference)
"""DualHOILoss Trainium2 kernel v2 (8 NeuronCores, data parallel over batch).

Math (per batch b, point p, vert v):
    u_p = basis_p + delta_p,  t_p = u_p/s + m,  w_v = v - m
    d2[p,v] = |t_p - v|^2 = u_p.r_v + c3*q_v + |u_p|^2/s^2
with r_v = -2*w_v/s, q_v = |r_v|^2, c3 = s^2/4.  One K=8 f32r matmul per
128-point tile: lhsT = cols 128t:128t+128 of a host-packed [8, 4096]
stack [dx,dy,dz,c3, bx,by,bz,0] (delta and basis transposed on the host,
pure layout), rhs[8, 778] rows [r;q;r;q] so the PE accumulates
u.r = d.r + b.r in one pass with no per-tile lhsT builds.  The rhs is
built per batch in vert-natural layout with DOUBLED chunks
[rx ry rz q rx ry rz q] so the PE transposes emit [8, 128] blocks
directly (no row-duplication DMA).

Anchor selection avoids one-hot masking: a tiny bf16 matmul per tile
with lhsT = host-packed one-hot [32, 128] and rhs = (a - m) [32, 3]
gathers each point's anchor g' = a_idx - m into PSUM; d2_sel =
|u/s - g'|^2 is then per-point elementwise work.  The 10 tail verts get
their own small matmuls; both run at batch start into the scratch PSUM
tile and are consumed early.

Each 768-col PSUM tile has exactly ONE reader (the tile framework
serializes same-tile PSUM readers): 15 odd tiles -> DVE
tensor_reduce(min) straight from PSUM; 17 tiles -> ACT drain (bf16) +
gpsimd tensor_tensor_scan(min,min) fold.  Batch stage combines mins,
adds |u|^2/s^2, computes d2_sel; squared-error sums are built by
gpsimd squares + DVE add-reduces into per-batch partial columns, and
the host sums the [128, 4] partials (and across cores).  ACT tails use
one exp-table load up front and one sqrt-table switch at the end.
"""

import numpy as np

B, P, A, V = 16, 4096, 32, 778
NCORES = 8
BPC = B // NCORES      # batches per core
NT = P // 128          # 32 point tiles per batch
PW = 768               # PSUM chunk-vert cols per tile
DW = V + 6             # drained cols per even tile (verts+tail+2x g = 784)
HALF = V // 2          # scan half width for drained tiles (389)
NTA = NT // 2          # even tiles on the ACT-drain path
INF = 3.0e38
NVC = 6                # full 128-vert chunks
VT = V - 128 * NVC     # tail verts (10)

_CACHE = {}


def _build_program():
    import concourse.bacc as bacc
    import concourse.mybir as mybir
    from concourse import tile

    f32 = mybir.dt.float32
    f32r = mybir.dt.float32r
    bf16 = mybir.dt.bfloat16
    u16 = mybir.dt.uint16
    AF = mybir.ActivationFunctionType
    ALU = mybir.AluOpType
    AX = mybir.AxisListType

    nc = bacc.Bacc(None, target_bir_lowering=False)

    chc_d = nc.dram_tensor("chc", [BPC, 128, 7 * NT], f32, kind="ExternalInput")
    udb_d = nc.dram_tensor("udb", [BPC, 8, P], f32r, kind="ExternalInput")
    baspm_d = nc.dram_tensor("baspm", [128, 3 * NT], f32, kind="ExternalInput")
    obj_d = nc.dram_tensor("obj", [BPC, 128, 24], f32, kind="ExternalInput")
    oht_d = nc.dram_tensor("oht", [BPC, 32, P], u16, kind="ExternalInput")
    sbc_d = nc.dram_tensor("sbc", [128, 16], f32, kind="ExternalInput")
    out_d = nc.dram_tensor("partials", [128, 4], f32, kind="ExternalOutput")

    cpack = np.eye(128, dtype=np.float32)
    cpack_d = nc.inline_tensor(cpack, "cpack")

    with tile.TileContext(nc) as tc:
        with (
            tc.tile_pool(name="sb", bufs=1) as sb,          # persistent
            tc.tile_pool(name="sb2", bufs=2) as sb2,        # per-batch
            tc.tile_pool(name="pp", bufs=4, space="PSUM") as ppool,
        ):
            sbc = sb.tile([128, 16], f32, tag="sbc")
            nc.sync.dma_start(sbc[:], sbc_d[:])
            cpk = sb.tile([128, 128], f32, tag="cpk")
            nc.scalar.dma_start(cpk[:], cpack_d[:])
            ident = cpk[:, 0:128]
            baspm = sb.tile([128, 3 * NT], f32, tag="baspm")
            nc.sync.dma_start(baspm[:], baspm_d[:])
            part = sb.tile([128, 4], f32, tag="part")

            neg2s = sbc[:, 6:7]
            inv_s2 = sbc[:, 7:8]
            inv_s = sbc[:, 8:9]
            poseps = sbc[:, 9:10]

            # warm the gpsimd library and the exp-capable ACT table at t=0
            # (no DMA dependency)
            warm = sb.tile([128, 2], f32, tag="warm")
            nc.gpsimd.memset(warm[:, 0:1], 0.0)
            nc.scalar.activation(warm[:, 1:2], warm[:, 0:1], AF.Exp)

            dsel2_l, ch_l = [], []

            for b in range(BPC):
                # ---------------- per-batch loads (obj first) ----------------
                obj = sb2.tile([128, 24], f32, tag="obj")
                nc.sync.dma_start(obj[:], obj_d[b])
                chc = sb2.tile([128, 7 * NT], f32, tag="chc")
                nc.sync.dma_start(chc[:], chc_d[b])
                ch = chc[:, 0 : 6 * NT]
                hc = chc[:, 6 * NT :]
                udb = sb2.tile([8, P], f32r, tag="udb")
                nc.sync.dma_start(udb[:], udb_d[b])
                oht = sb2.tile([32, P], bf16, tag="oht")
                nc.sync.dma_start(oht[:].bitcast(u16), oht_d[b])

                mneg = sbc[:, 3 * b : 3 * b + 3]
                mraw = sbc[:, 10 + 3 * b : 13 + 3 * b]

                # ---------------- rhs build (doubled chunks) ----------------
                # rq: per chunk cols [r_x, r_y, r_z, q] x2 so PE transposes
                # produce the K=8 [r;q;r;q] blocks directly
                rq = sb2.tile([128, 64], f32, tag="rq")
                rqv = rq[:].rearrange("p (c u d) -> p c u d", u=2, d=4)
                w24 = sb2.tile([128, 24], f32, tag="w24")
                nc.gpsimd.tensor_tensor(
                    w24[:].rearrange("p (c d) -> p c d", d=3),
                    obj[:].rearrange("p (c d) -> p c d", d=3),
                    mraw.unsqueeze(1).broadcast_to([128, 8, 3]),
                    op=ALU.subtract,
                )
                for u in range(2):
                    nc.gpsimd.tensor_tensor(
                        rqv[:, :, u, 0:3],
                        w24[:].rearrange("p (c d) -> p c d", d=3),
                        neg2s.unsqueeze(1).broadcast_to([128, 8, 3]),
                        op=ALU.mult,
                    )
                rsq = sb2.tile([128, 24], f32, tag="rsq")
                nc.gpsimd.tensor_tensor(
                    rsq[:].rearrange("p (c d) -> p c d", d=3),
                    rqv[:, :, 0, 0:3],
                    rqv[:, :, 0, 0:3],
                    op=ALU.mult,
                )
                nc.vector.tensor_reduce(
                    rqv[:, :, 0, 3:4].squeeze(2),
                    rsq[:].rearrange("p (c d) -> p c d", d=3),
                    axis=AX.X, op=ALU.add,
                )
                nc.vector.tensor_copy(
                    rqv[:, :, 1, 3:4].squeeze(2), rqv[:, :, 0, 3:4].squeeze(2))
                # gathered-anchor rhs: (a - m), 3 cols, bf16
                wan = sb2.tile([32, 4], bf16, tag="wan")
                nc.gpsimd.tensor_tensor(
                    wan[:, 0:3], obj[0:A, 21:24], mraw[0:A], op=ALU.subtract)

                scr = ppool.tile([128, 1024], f32, tag="pp")
                for c in range(NVC):
                    nc.tensor.transpose(
                        scr[0:8, 128 * c : 128 * (c + 1)],
                        rq[:, 8 * c : 8 * c + 8], ident)
                nc.tensor.transpose(
                    scr[0:8, 768:778], rq[0:VT, 48:56], ident[0:VT, 0:VT])
                rhs = sb2.tile([8, 896], f32r, tag="rhs")
                nc.scalar.activation(rhs[0:8, 0:V], scr[0:8, 0:V], AF.Copy)

                # ---------------- u point-major + |u|^2 ----------------
                upm = sb2.tile([128, 3 * NT], f32, tag="upm")
                nc.gpsimd.tensor_tensor(
                    upm[:].rearrange("p (t d) -> p t d", d=3),
                    ch[:].rearrange("p (t s) -> p t s", s=6)[:, :, 1:4],
                    baspm[:].rearrange("p (t d) -> p t d", d=3),
                    op=ALU.add,
                )
                usq = sb2.tile([128, 3 * NT], f32, tag="usq")
                nc.gpsimd.tensor_tensor(usq[:], upm[:], upm[:], op=ALU.mult)
                uu = sb2.tile([128, NT], f32, tag="uu")
                nc.vector.tensor_reduce(
                    uu[:], usq[:].rearrange("p (t d) -> p t d", d=3),
                    axis=AX.X, op=ALU.add,
                )

                # ---------------- main tile loop ----------------
                # one PSUM reader per tile: odd t -> DVE reduce straight from
                # PSUM (verts+tail); even t -> ACT drain (bf16, incl tail and
                # both tiles' gathered-anchor cols) + gpsimd scan fold
                m2all = sb2.tile([128, NT], f32, tag="m2all")
                drain = sb2.tile([128, (NTA + 8) * DW], bf16, tag="drain")
                drv = drain[:].rearrange("p (t w) -> p t w", w=DW)
                junk = sb2.tile([128, (NTA + 8) * HALF], bf16, tag="junk")
                jv = junk[:].rearrange("p (t w) -> p t w", w=HALF)

                for t in range(NT):
                    pp = ppool.tile([128, 1024], f32, tag="pp")
                    # matmul outs must stay inside 512-col PSUM banks
                    for lo, hi in ((0, 512), (512, PW), (PW, V)):
                        nc.tensor.matmul(
                            pp[:, lo:hi],
                            udb[:, 128 * t : 128 * (t + 1)],
                            rhs[:, lo:hi],
                            start=True, stop=True)
                    if t % 2 == 0:
                        # this tile's (and a DVE-path partner's) anchor gather
                        for u in range(2):
                            nc.tensor.matmul(
                                pp[:, V + 3 * u : V + 3 * u + 3],
                                oht[:, 128 * (t + u) : 128 * (t + u + 1)],
                                wan[:, 0:3],
                                start=True, stop=True)
                    elif t % 4 == 3:
                        nc.tensor.matmul(
                            pp[:, V : V + 3],
                            oht[:, 128 * t : 128 * (t + 1)],
                            wan[:, 0:3],
                            start=True, stop=True)
                    if t % 4 == 1:
                        nc.vector.tensor_reduce(
                            m2all[:, t : t + 1], pp[:, 0:V],
                            axis=AX.X, op=ALU.min,
                        )
                    else:
                        ta = t // 2 if t % 2 == 0 else 16 + t // 4
                        w = DW if t % 2 == 0 else V + 3
                        nc.scalar.activation(
                            drv[:, ta, 0:w], pp[:, 0:w], AF.Copy)
                    # fold a drained tile from 6 tiles back: its drain is
                    # long done, so the in-order DVE queue never stalls
                    tl = t - 6
                    if tl >= 0 and tl % 4 != 1:
                        tal = tl // 2 if tl % 2 == 0 else 16 + tl // 4
                        nc.vector.tensor_tensor_scan(
                            out=jv[:, tal, :],
                            data0=drv[:, tal, 0:HALF],
                            data1=drv[:, tal, HALF:V],
                            initial=INF, op0=ALU.min, op1=ALU.min,
                        )
                for tl in range(NT - 6, NT):
                    if tl % 4 != 1:
                        tal = tl // 2 if tl % 2 == 0 else 16 + tl // 4
                        nc.vector.tensor_tensor_scan(
                            out=jv[:, tal, :],
                            data0=drv[:, tal, 0:HALF],
                            data1=drv[:, tal, HALF:V],
                            initial=INF, op0=ALU.min, op1=ALU.min,
                        )

                # ---------------- batch stage ----------------
                # scan tails -> even cols + t%4==3 cols of m2all
                nc.gpsimd.tensor_copy(
                    m2all[:].rearrange("p (t two) -> p t two", two=2)
                        [:, :, 0:1].squeeze(2),
                    jv[:, 0:NTA, HALF - 1],
                )
                nc.gpsimd.tensor_copy(
                    m2all[:].rearrange("p (q f) -> p q f", f=4)
                        [:, :, 3:4].squeeze(2),
                    jv[:, NTA : NTA + 8, HALF - 1],
                )
                mind2 = sb2.tile([128, NT], f32, tag="mind2")
                nc.vector.scalar_tensor_tensor(
                    out=mind2[:], in0=uu[:], scalar=inv_s2, in1=m2all[:],
                    op0=ALU.mult, op1=ALU.add,
                )
                gd = sb2.tile([128, 3 * NT], f32, tag="gd")
                ts_ = sb2.tile([128, 3 * NT], f32, tag="tscale")
                nc.gpsimd.tensor_tensor(
                    ts_[:], upm[:],
                    inv_s.broadcast_to([128, 3 * NT]),
                    op=ALU.mult,
                )
                nc.gpsimd.tensor_tensor(
                    gd[:].rearrange("p (k ud) -> p k ud", ud=6)[:, :, 0:3],
                    ts_[:].rearrange("p (k ud) -> p k ud", ud=6)[:, :, 0:3],
                    drv[:, 0:NTA, V : V + 3],
                    op=ALU.subtract,
                )
                nc.gpsimd.tensor_tensor(
                    gd[:].rearrange("p (q f) -> p q f", f=12)[:, :, 3:6],
                    ts_[:].rearrange("p (q f) -> p q f", f=12)[:, :, 3:6],
                    drv[:, 0:NTA:2, V + 3 : V + 6],
                    op=ALU.subtract,
                )
                nc.gpsimd.tensor_tensor(
                    gd[:].rearrange("p (q f) -> p q f", f=12)[:, :, 9:12],
                    ts_[:].rearrange("p (q f) -> p q f", f=12)[:, :, 9:12],
                    drv[:, NTA : NTA + 8, V : V + 3],
                    op=ALU.subtract,
                )
                gd2 = sb2.tile([128, 3 * NT], f32, tag="gd2")
                nc.gpsimd.tensor_tensor(gd2[:], gd[:], gd[:], op=ALU.mult)
                dsel2 = sb2.tile([128, NT], f32, tag="dsel2")
                nc.vector.tensor_reduce(
                    dsel2[:], gd2[:].rearrange("p (t d) -> p t d", d=3),
                    axis=AX.X, op=ALU.add,
                )

                # contact tail (exp table still loaded; overlaps next batch)
                cont = sb2.tile([128, NT], f32, tag="cont")
                nc.scalar.activation(cont[:], mind2[:], AF.Exp, scale=-100.0)
                cdiff = sb2.tile([128, NT], f32, tag="cdiff")
                nc.gpsimd.tensor_tensor(
                    cdiff[:], cont[:], hc[:], op=ALU.subtract)
                cdiff2 = sb2.tile([128, NT], f32, tag="cdiff2")
                nc.gpsimd.tensor_tensor(
                    cdiff2[:], cdiff[:], cdiff[:], op=ALU.mult)
                nc.vector.tensor_reduce(
                    part[:, 2 + b : 3 + b], cdiff2[:], axis=AX.X, op=ALU.add)

                dsel2_l.append(dsel2)
                ch_l.append(ch)

            # ---------------- end phase: sqrt side ----------------
            for b in range(BPC):
                dsel = sb.tile([128, NT], f32, tag=f"dsel{b}")
                nc.scalar.activation(dsel[:], dsel2_l[b][:], AF.Sqrt,
                                     bias=poseps)
                ddiff = sb.tile([128, NT], f32, tag=f"ddiff{b}")
                nc.gpsimd.tensor_tensor(
                    ddiff[:], dsel[:],
                    ch_l[b][:].rearrange("p (t s) -> p t s", s=6)[:, :, 4:5]
                        .squeeze(2),
                    op=ALU.subtract,
                )
                ddiff2 = sb.tile([128, NT], f32, tag=f"dd2{b}")
                nc.gpsimd.tensor_tensor(
                    ddiff2[:], ddiff[:], ddiff[:], op=ALU.mult)
                nc.vector.tensor_reduce(
                    part[:, b : b + 1], ddiff2[:], axis=AX.X, op=ALU.add)

            nc.sync.dma_start(out_d[:], part[:])

    nc.compile()
    return nc


def _get_program():
    if "nc" not in _CACHE:
        _CACHE["nc"] = _build_program()
    return _CACHE["nc"]


def _host_prep(verts, anchors, choir, hand_contacts, bps_mean, bps_scalar,
               bps_basis):
    verts = np.ascontiguousarray(np.asarray(verts, np.float32))
    anchors = np.ascontiguousarray(np.asarray(anchors, np.float32))
    choir = np.ascontiguousarray(np.asarray(choir, np.float32))
    hand_contacts = np.ascontiguousarray(np.asarray(hand_contacts, np.float32))
    bps_mean = np.ascontiguousarray(np.asarray(bps_mean, np.float32))
    s = np.float32(np.asarray(bps_scalar).reshape(()))
    basis = np.ascontiguousarray(np.asarray(bps_basis, np.float32))
    c3 = np.float32(s * s / 4.0)

    chc = np.concatenate(
        [choir.reshape(B, 128, 6 * NT), hand_contacts.reshape(B, 128, NT)],
        axis=2)
    # K=8 lhsT stack: col 128t+j = point 32j+t; rows [dx,dy,dz,c3,bx,by,bz,0]
    ch4 = choir[:, :, 1:4].reshape(B, 128, NT, 3).transpose(0, 3, 2, 1)
    bas3 = basis.reshape(128, NT, 3).transpose(2, 1, 0)  # [3, t, j]
    udb = np.empty((B, 8, NT, 128), np.float32)
    udb[:, 0:3] = ch4                      # [B, 3, t, j]
    udb[:, 3] = c3
    udb[:, 4:7] = bas3[None]
    udb[:, 7] = 0.0
    udb = np.ascontiguousarray(udb.reshape(B, 8, P))
    baspm = np.ascontiguousarray(basis.reshape(128, 3 * NT))

    obj = np.zeros((B, 128, 24), np.float32)
    obj[:, :, 0:18] = verts[:, 0:768, :].reshape(B, NVC, 128, 3).transpose(
        0, 2, 1, 3).reshape(B, 128, 18)
    obj[:, 0:VT, 18:21] = verts[:, 768:V, :]
    obj[:, 0:A, 21:24] = anchors

    idx = choir[:, :, 5].astype(np.int32).reshape(B, 128, NT)
    eq = (idx.transpose(0, 2, 1)[:, :, :, None]
          == np.arange(A, dtype=np.int32)).astype(np.float32)  # [B,t,j,a]
    oht32 = np.ascontiguousarray(eq.transpose(0, 3, 1, 2).reshape(B, A, P))
    oht = (oht32.view(np.uint32) >> 16).astype(np.uint16)  # exact bf16 bits

    in_maps = []
    for c in range(NCORES):
        lo = BPC * c
        row = np.zeros(16, np.float32)
        for bb in range(BPC):
            m = bps_mean[lo + bb].reshape(3)
            row[3 * bb : 3 * bb + 3] = m * (np.float32(-2.0) / s)
            row[10 + 3 * bb : 13 + 3 * bb] = m
        row[6] = np.float32(-2.0) / s
        row[7] = np.float32(1.0) / (s * s)
        row[8] = np.float32(1.0) / s
        row[9] = 1.0e-12
        in_maps.append({
            "chc": chc[lo : lo + BPC],
            "udb": udb[lo : lo + BPC],
            "baspm": baspm,
            "obj": obj[lo : lo + BPC],
            "oht": oht[lo : lo + BPC],
            "sbc": np.tile(row, (128, 1)),
        })
    return in_maps


def kernel(verts, anchors, choir, hand_contacts, bps_mean, bps_scalar,
           bps_basis, _trace=False):
    from concourse.bass_utils import run_bass_kernel_spmd

    in_maps = _host_prep(verts, anchors, choir, hand_contacts, bps_mean,
                         bps_scalar, bps_basis)
    nc = _get_program()
    res = run_bass_kernel_spmd(nc, in_maps, list(range(NCORES)), trace=_trace)
    parts = np.stack([np.asarray(r["partials"], np.float64).reshape(128, 4)
                      .sum(axis=0) for r in res.results])
    choir_loss = parts[:, 0:BPC].sum() / (B * P)
    contact_loss = parts[:, 2 : 2 + BPC].sum() / (B * P)
    out = (np.float32(choir_loss), np.float32(contact_loss))
    if _trace:
        return out, res
    return out
